# revision 35
# baseline (speedup 1.0000x reference)
"""Trainium2 Bass kernel for nn_CaC_Module (dynamic-kernel dilated depthwise CNN).

Per-sample computation (b=8 sharded 1/core across 8 NeuronCores):
  query = Wq @ x          (1x1 conv, [9, hw])
  q     = softmax(query over hw)          (bq cancels in softmax -> ignored)
  kern  = Wk @ (x @ q^T) + bk             (associativity: avoids the big
                                           key GEMM entirely; bk folds in
                                           because sum_n q = 1)
  out   = x * sum_d sigmoid(depthwise3x3(x, kern, dil=d)),  d in (1,3,5)

Conv phase (unchanged from the 149us version -- it is at the fp8 DR
hardware floor, ~2.46us/window with every engine 78-100% busy):
  - Depthwise conv = accumulating DR matmuls with diagonal stationary
    weights over a flat padded layout (row stride 69); 8 outer taps as 4
    fp8 DoubleRow passes per (cb, dil, window); fp16 center tap split
    DVE-STT / PE; per-dilation psum tiles from a 6-bank ring; tails
    software-pipelined one window late.

Front phase (rebuilt, ~47us -> ~22us):
  - x8 DMA in 5 row-chunks, each ONE combined all-cb transfer (small
    transfers pay ~1us serial queue overhead each), alternating the
    sync/scalar HW queues; query window w is gated only on its chunk
    (sub-tile deps).
  - Raw query drained margin-free, transposed in chunk pairs, then ONE
    exp ACT per 8-chunk group (replaces 10 serial per-window exps).
  - Softmax denominator rides a ones-column appended to xT8, so esum
    falls out of the G^T GEMM; G^T accumulates chunk-pairs as they land.
  - kern GEMM pipelined against the G transposes, with one psum tile per
    output block in SEPARATE banks: interleaved matmul accumulation
    groups corrupt each other within a single psum bank (hw-verified).
  - xT8+wkT ride the gpsimd queue in parallel with x8.
Post-schedule pass drops InstLdweights whose weights AP repeats the
previous load (warmup spins, G^T ch pairs) -- minor queue-slot win.
"""
import numpy as np

C, H, W = 512, 64, 64
P, CB = 128, 4
RS = 69                   # row stride: 64 data + 5 shared zero margin
HEAD = 5                  # guard zeros before row 0
VPAD = 5                  # zero rows above/below the image
XLEN = 5120               # per-(channel,cb) flat buffer length
RT = 7                    # image rows per conv/query window
NW = 10                   # 9 windows x 7 rows + 1 window x 1 row = 64 rows
NCH = 32                  # margin-free n-chunks of 128 (4096 data cols)
XTW = 257                 # xT8 cols per channel-half: 256 data + 1 ones-col
RATES = (1, 3, 5)
PAIRS = ((0, 1), (2, 3), (5, 6), (7, 8))   # DR tap pairs; tap 4 = center
NCORES = 8

_CACHE = {}


def _flat(r, x):
    # buffer index of image row r (may be in [-5, 69)), column x
    return HEAD + (VPAD + r) * RS + x


def _dedup_ldweights(nc):
    """Remove InstLdweights whose weights AP is identical to the previous
    (kept) InstLdweights on the same engine block with only InstMatmult in
    between; the following MATMULs reuse the loaded array state."""
    inbound = set()
    for f in nc.m.functions:
        for blk in f.blocks:
            for i in blk.instructions:
                inbound.update(i.sync_dependency_names())
                inbound.update(i.nosync_dependency_names())
    nrem = 0
    for f in nc.m.functions:
        for blk in f.blocks:
            insts = list(blk.instructions)
            prev_sig = None
            to_remove = []
            for idx, inst in enumerate(insts):
                nm = type(inst).__name__
                if nm == 'InstLdweights':
                    sig = (str(inst.ins[0]), str(inst.perf_mode),
                           str(inst.is_transpose), str(inst.tile_position),
                           str(inst.tile_size))
                    if (sig == prev_sig and inst.name not in inbound
                            and idx + 1 < len(insts)
                            and type(insts[idx + 1]).__name__ == 'InstMatmult'):
                        nxt = insts[idx + 1]
                        nxt.add_sync_dependencies_from(
                            inst.sync_dependency_set_copy())
                        nxt.add_nosync_dependencies_from(
                            inst.nosync_dependency_set_copy())
                        to_remove.append(inst)
                    else:
                        prev_sig = sig
                elif nm == 'InstMatmult':
                    pass          # does not clobber loaded weights
                else:
                    prev_sig = None
            for r in to_remove:
                blk.instructions.remove(r)
            nrem += len(to_remove)
    return nrem


def _build_program():
    import concourse.bacc as bacc
    import concourse.bass as bass
    import concourse.mybir as mybir
    from concourse.tile import TileContext

    dt = mybir.dt
    AF = mybir.ActivationFunctionType
    ALU = mybir.AluOpType
    DR = mybir.MatmulPerfMode.DoubleRow
    f32, f16, f8 = dt.float32, dt.float16, dt.float8e4

    nc = bacc.Bacc()
    x8_d = nc.declare_dram_parameter("x8", [C, XLEN], f8, isOutput=False)
    xf_d = nc.declare_dram_parameter("xf", [C, XLEN], f16, isOutput=False)
    xT8_d = nc.declare_dram_parameter("xT8", [P, 2 * NCH * XTW], f8,
                                      isOutput=False)
    wkT_d = nc.declare_dram_parameter("wkT", [C, C], f16, isOutput=False)
    wq8T_d = nc.declare_dram_parameter("wq8T", [C, 16], f8, isOutput=False)
    bk_d = nc.declare_dram_parameter("bk", [C], f32, isOutput=False)
    id9h_d = nc.declare_dram_parameter("id9h", [9, 9], f16, isOutput=False)
    id8_d = nc.declare_dram_parameter("id8", [P, P], f8, isOutput=False)
    out_d = nc.declare_dram_parameter("out", [C, H, W], f16, isOutput=True)

    def winsize(w):
        return (RT * RS) if w < NW - 1 else RS  # 483 or 69

    def nrows(w):
        return RT if w < NW - 1 else 1

    def pair_ap(win, delta):
        # [P, N] window -> [P, 2, N]: second k-tile shifted by delta elems
        return bass.AP(tensor=win.tensor, offset=win.offset,
                       ap=[list(win.ap[0]), [delta, 2], list(win.ap[-1])])

    # x8 chunk boundaries: chunk k covers query windows 2k, 2k+1 exactly
    BND = [0] + [_flat(14 * k, 0) for k in (1, 2, 3, 4)] + [XLEN]

    with TileContext(nc) as tc:
        with (
            tc.tile_pool(name="const", bufs=1) as cpool,
            tc.tile_pool(name="diagp", bufs=2) as dpool,
            tc.tile_pool(name="sigp", bufs=4) as sigp,
            tc.tile_pool(name="tmpp", bufs=3) as tmpp,
            tc.tile_pool(name="outp", bufs=3) as opool,
        ):
            psA = tc.alloc_tile_pool(name="psA", bufs=2, space="PSUM")
            psG = tc.alloc_tile_pool(name="psG", bufs=1, space="PSUM")
            x8 = cpool.tile([P, CB, XLEN], f8)
            xf = cpool.tile([P, CB, XLEN], f16)
            xT8 = cpool.tile([P, 2, NCH, XTW], f8)
            wkT = cpool.tile([P, CB, C], f16)
            wq8T = cpool.tile([P, CB, 16], f8)
            bk = cpool.tile([P, CB], f32)
            id9h = cpool.tile([9, 9], f16)
            id8 = cpool.tile([P, P], f8)
            query = cpool.tile([32, 4096], f16)   # rows 9..31 unused
            qTf = cpool.tile([P, NCH, 16], f16)   # cols 9..15 junk
            qT8 = cpool.tile([P, NCH, 16], f8)
            rinv = cpool.tile([9, 1], f32)
            gs = cpool.tile([9, C], f16)
            G = cpool.tile([P, CB, 9], f16)
            kern = cpool.tile([P, CB, 9], f32)

            # ---- DMA routing.  The gpsimd SW-DGE queue streams at full
            # rate from t=0 while the sync/scalar HW-DGE queues crawl for
            # the first ~15us (hw ramp): the conv-critical bytes
            # (x8 -> xT8ch0) ride gpsimd; xT8ch1 rides scalar; wkT + xf
            # ride sync; tiny params go first on scalar. ----
            HB = _flat(28, 0)       # x8 half boundary: windows 0-3 | 4-9
            DLO, DHI = _flat(0, 0), _flat(64, 0)   # x8 data-row extent
            nc.scalar.dma_start(out=id8[:], in_=id8_d[:])
            nc.scalar.dma_start(out=wq8T[:], in_=wq8T_d[:].rearrange(
                "(cb p) t -> p cb t", p=P))
            nc.scalar.dma_start(out=id9h[:], in_=id9h_d[:])
            nc.scalar.dma_start(
                out=bk[:], in_=bk_d[:].rearrange("(cb p) -> p cb", p=P))
            # x8's vertical zero pads are never DMA'd: memset them (the
            # conv's dy-shifted windows read them).  Per-cb half transfers
            # keep the sub-tile dependency boxes tight so query window w
            # waits only for the halves it reads.
            nc.vector.memset(x8[:, :, 0:DLO], 0.0)
            nc.vector.memset(x8[:, :, DHI:XLEN], 0.0)
            for a, b in ((DLO, HB), (HB, DHI)):
                nc.gpsimd.dma_start(
                    out=x8[:, :, a:b],
                    in_=x8_d[:, a:b].rearrange("(cb p) n -> p cb n", p=P))
            for g in range(4):
                nc.gpsimd.dma_start(
                    out=xT8[:, 0, 8 * g:8 * (g + 1), :],
                    in_=xT8_d[:, 8 * g * XTW:8 * (g + 1) * XTW])
            nc.sync.dma_start(
                out=wkT[:], in_=wkT_d[:].rearrange("(cb p) o -> p cb o", p=P))
            for g in range(2):
                nc.sync.dma_start(
                    out=xT8[:, 1, 16 * g:16 * (g + 1), :],
                    in_=xT8_d[:, (NCH + 16 * g) * XTW:(NCH + 16 * (g + 1)) * XTW])

            # qTf junk cols see exp() later: zero them once
            nc.vector.memset(qTf[:], 0.0)

            # ---- PE warmup: dummy matmuls on a memset tile open the HAM
            # clock-gate before the first real (DMA-gated) matmul ----
            wz = cpool.tile([P, 512], f8)
            nc.vector.memset(wz[:], 0.0)
            # spins bridge until x8 lands (~20us) so the whole front matmul
            # chain (queries/transposes/G^T/kern) runs at full p-state
            # instead of the half-clock the PE drops to after ~8us idle
            pw = psA.tile([P, 512], f32, tag="ps", bufs=4)
            for i in range(90):
                nc.tensor.matmul(pw[:], lhsT=wz[:, 0:P], rhs=wz[:],
                                 start=(i == 0), stop=(i == 89))

            # ---- G^T psums: [9, 257]; col 256 = sum(exp) (ones-column) ----
            pgt = [psG.tile([9, XTW], f32, tag=f"gt{ch}", name=f"pgt{ch}")
                   for ch in range(2)]

            state = {"np": 0, "g": 0}

            def pump(cols_avail):
                # transposes of ready chunk-pairs
                while state["np"] < NCH // 2 and \
                        256 * (state["np"] + 1) <= cols_avail:
                    npi = state["np"]
                    pst2 = psA.tile([P, 2, 16], f16, tag="pt")
                    for j in range(2):
                        nc.tensor.transpose(
                            pst2[:, j, 0:9],
                            query[0:9, (2 * npi + j) * P:(2 * npi + j + 1) * P],
                            id9h[:])
                    nc.vector.tensor_copy(
                        qTf[:, 2 * npi:2 * npi + 2, 0:9], pst2[:, :, 0:9])
                    state["np"] += 1
                # exp per 8-chunk group + its G^T passes
                while state["g"] < 4 and state["np"] >= 4 * (state["g"] + 1):
                    g = state["g"]
                    nc.scalar.activation(
                        qT8[:, 8 * g:8 * (g + 1), :],
                        qTf[:, 8 * g:8 * (g + 1), :], AF.Exp)
                    for npi in range(4 * g, 4 * (g + 1)):
                        for ch in range(2):
                            nc.tensor.matmul(
                                pgt[ch][:],
                                lhsT=qT8[:, 2 * npi:2 * npi + 2, 0:9],
                                rhs=xT8[:, ch, 2 * npi:2 * npi + 2, :],
                                start=(npi == 0), stop=(npi == NCH // 2 - 1),
                                perf_mode=DR, skip_group_check=True)
                    state["g"] += 1

            # ---- query windows; window w is gated (sub-tile deps) only on
            # the x8 half covering it ----
            def emit_query(w):
                N = winsize(w)
                nr = nrows(w)
                base = _flat(RT * w, 0)
                psq = psA.tile([9, N], f32, tag="ps", bufs=4)
                for kc in range(0, CB, 2):
                    nc.tensor.matmul(
                        psq[:],
                        lhsT=wq8T[:, kc:kc + 2, 0:9],
                        rhs=pair_ap(x8[:, kc, base:base + N], XLEN),
                        start=(kc == 0), stop=(kc == 2),
                        perf_mode=DR)
                # drain data cols (margin-free) to fp16 SBUF; alternate
                # Scalar/Vector so the drain chain is not serial on one
                # engine (GpSimd cannot read PSUM)
                if w % 2 == 0:
                    nc.scalar.copy(
                        query[0:9, w * 448:w * 448 + nr * W].rearrange(
                            "p (r c) -> p r c", c=W),
                        psq[:].rearrange("p (r c) -> p r c", c=RS)[:, :, 0:W])
                else:
                    nc.vector.tensor_copy(
                        query[0:9, w * 448:w * 448 + nr * W].rearrange(
                            "p (r c) -> p r c", c=W),
                        psq[:].rearrange("p (r c) -> p r c", c=RS)[:, :, 0:W])
                pump(448 * (w + 1))

            for w in range(NW):
                emit_query(w)

            # ---- xf cb0 (sync queue, behind xT8ch1): head first so the
            # first conv tails are never starved; pads never read -> not
            # loaded.  cb1-3 are deferred into the conv loop so they do
            # not steal early DMA bandwidth from x8/xT8. ----
            nc.sync.dma_start(out=xf[:, 0, DLO:HB], in_=xf_d[0:P, DLO:HB])
            nc.sync.dma_start(out=xf[:, 0, HB:DHI], in_=xf_d[0:P, HB:DHI])

            pump(4096)   # flush leftovers (no-op if chunk loop covered all)

            # ---- softmax denominator + normalized G^T (gs) ----
            nc.vector.reciprocal(rinv[:], pgt[0][:, 256:257])
            for ch in range(2):
                nc.vector.tensor_scalar_mul(
                    gs[:, ch * 256:(ch + 1) * 256], pgt[ch][:, 0:256], rinv[:])

            # ---- G = gs^T; kern = Wk @ G + bk, pipelined per ci-pair.
            # One psum tile per co in SEPARATE banks: interleaved matmul
            # accumulation groups corrupt each other within a bank. ----
            psns = [psA.tile([P, 9], f32, tag="ps", bufs=4, name=f"psn{co}")
                    for co in range(CB)]
            for cp in range(2):
                psx = psA.tile([P, 2, 16], f16, tag="pt")
                for j in range(2):
                    ci = 2 * cp + j
                    nc.tensor.transpose(
                        psx[:, j, 0:9], gs[:, ci * P:(ci + 1) * P], id9h[:])
                nc.vector.tensor_copy(
                    G[:, 2 * cp:2 * cp + 2, :], psx[:, :, 0:9])
                for j in range(2):
                    ci = 2 * cp + j
                    for co in range(CB):
                        nc.tensor.matmul(
                            psns[co][:],
                            lhsT=wkT[:, ci, co * P:(co + 1) * P],
                            rhs=G[:, ci, :],
                            start=(ci == 0), stop=(ci == CB - 1),
                            skip_group_check=True)
            for co in range(CB):
                nc.vector.tensor_scalar_add(
                    kern[:, co], psns[co][:], bk[:, co:co + 1])

            # ---- depthwise convs: fp8 DR tap pairs on PE + fp16 center ----
            TAPS9 = (0, 1, 2, 3, 5, 6, 7, 8, 4)
            diag8s = [dpool.tile([P, 9, P], f8, tag=f"d8_{cb}",
                                 name=f"diag8_{cb}") for cb in range(CB)]
            diag16s = [dpool.tile([P, P], f16, tag=f"d16_{cb}",
                                  name=f"diag16_{cb}") for cb in range(CB)]

            def emit_diag(cb, t, on_vector):
                if t == 4:
                    nc.vector.tensor_scalar_mul(
                        diag16s[cb][:], id8[:], kern[:, cb, 4:5])
                elif on_vector:
                    nc.vector.tensor_scalar_mul(
                        diag8s[cb][:, t], id8[:], kern[:, cb, t:t + 1])
                else:
                    nc.scalar.mul(
                        diag8s[cb][:, t], id8[:], kern[:, cb, t:t + 1])

            # cb0's diags build at conv start, split Scalar/Vector; later
            # cbs' builds are drip-fed one per window of the previous cb
            for j, t in enumerate(TAPS9):
                emit_diag(0, t, on_vector=(j % 2 == 1))

            psG.release()
            psA.release()
            psD = tc.alloc_tile_pool(name="psD", bufs=6, space="PSUM")

            def emit_tail(cb, w, pds, fast=False, outq=None):
                # sigmoid + sums + final mul + store for a finished window;
                # emitted one window late so no engine queue head-blocks
                # behind the sigmoid.  fast=True (end-of-kernel drain) puts
                # the sums on the quicker DVE instead of GpSimd.
                N = winsize(w)
                nr = nrows(w)
                r0 = RT * w
                st = sigp.tile([P, 3, RT * W], f16, tag="sig")
                for di in range(3):
                    nc.scalar.activation(
                        st[:, di, 0:nr * W].rearrange(
                            "p (r c) -> p r c", c=W),
                        pds[di][:, 0:N].rearrange(
                            "p (r c) -> p r c", c=RS)[:, :, 0:W],
                        AF.Sigmoid)
                t01 = tmpp.tile([P, RT * W], f16, tag="t01")
                w3 = tmpp.tile([P, RT * W], f16, tag="w3")
                eng = nc.vector if fast else nc.gpsimd
                nc.vector.tensor_add(
                    t01[:, 0:nr * W], st[:, 0, 0:nr * W], st[:, 1, 0:nr * W])
                eng.tensor_add(
                    w3[:, 0:nr * W], t01[:, 0:nr * W], st[:, 2, 0:nr * W])
                ot = opool.tile([P, RT * W], f16, tag="ot")
                eng.tensor_mul(
                    ot[:, 0:nr * W].rearrange("p (r c) -> p r c", c=W),
                    w3[:, 0:nr * W].rearrange("p (r c) -> p r c", c=W),
                    xf[:, cb, _flat(r0, 0):_flat(r0, 0) + N]
                    .rearrange("p (r c) -> p r c", c=RS)[:, :, 0:W])
                q = outq or nc.sync
                if nr > 1:
                    hr = nr // 2
                    q.dma_start(
                        out=out_d[cb * P:(cb + 1) * P, r0:r0 + hr, :],
                        in_=ot[:, 0:hr * W].rearrange("p (r c) -> p r c", c=W))
                    q.dma_start(
                        out=out_d[cb * P:(cb + 1) * P, r0 + hr:r0 + nr, :],
                        in_=ot[:, hr * W:nr * W].rearrange("p (r c) -> p r c", c=W))
                else:
                    q.dma_start(
                        out=out_d[cb * P:(cb + 1) * P, r0:r0 + nr, :],
                        in_=ot[:, 0:nr * W].rearrange("p (r c) -> p r c", c=W))

            prev = None
            for cb in range(CB):
                for w in range(NW):
                    N = winsize(w)
                    r0 = RT * w
                    if cb == CB - 1 and w == NW - 1 and prev is not None:
                        # last pair: drain the full window w8 while w9's
                        # matmuls stream, so only w9's tiny tail trails
                        emit_tail(*prev, fast=True)
                        prev = None
                    # center taps: di 0,1 on DVE (STT), di 2 as an fp16 PE
                    # matmul -- balances PE vs DVE; the last windows of the
                    # last cb go all-PE to shorten the end-of-kernel drain
                    all_pe = (cb == CB - 1 and w >= NW - 2)
                    pds = []
                    for di, d in enumerate(RATES):
                        pdi = psD.tile([P, 512], f32, tag="pd",
                                       name=f"pd_{cb}_{w}_{di}")
                        pds.append(pdi)
                        offs = {}
                        for t in range(9):
                            dy, dx = t // 3 - 1, t % 3 - 1
                            offs[t] = _flat(r0 + dy * d, dx * d)
                        pe_center = all_pe
                        for pi, (t0, t1) in enumerate(PAIRS):
                            nc.tensor.matmul(
                                pdi[:, 0:N],
                                lhsT=diag8s[cb][:, t0:t0 + 2, :],
                                rhs=pair_ap(x8[:, cb, offs[t0]:offs[t0] + N],
                                            offs[t1] - offs[t0]),
                                start=(pi == 0),
                                stop=(pi == len(PAIRS) - 1 and not pe_center),
                                perf_mode=DR)
                        if pe_center:
                            nc.tensor.matmul(
                                pdi[:, 0:N],
                                lhsT=diag16s[cb][:],
                                rhs=xf[:, cb, offs[4]:offs[4] + N],
                                start=False, stop=True,
                                skip_group_check=True)
                        else:
                            nc.vector.scalar_tensor_tensor(
                                pdi[:, 0:N],
                                in0=xf[:, cb, offs[4]:offs[4] + N],
                                scalar=kern[:, cb, 4:5],
                                in1=pdi[:, 0:N],
                                op0=ALU.mult, op1=ALU.add)
                    # deferred xf streams (bandwidth freed for x8/xT8 early)
                    if cb == 0 and w in (0, 2, 4):
                        fcb = 1 + w // 2
                        nc.sync.dma_start(
                            out=xf[:, fcb, DLO:DHI],
                            in_=xf_d[fcb * P:(fcb + 1) * P, DLO:DHI])
                    # drip-feed next cb's diag builds (one per window)
                    if cb + 1 < CB and w < 9:
                        emit_diag(cb + 1, TAPS9[w], on_vector=False)
                    if prev is not None:
                        emit_tail(*prev)
                    prev = (cb, w, pds)
            emit_tail(*prev, fast=True)
            psD.release()
    _CACHE["ldw_removed"] = _dedup_ldweights(nc)
    nc.finalize()
    return nc


def _get_program():
    if "nc" not in _CACHE:
        _CACHE["nc"] = _build_program()
    return _CACHE["nc"]


def make_in_maps(x, Wk, bk, Wq, bq=None):
    import ml_dtypes
    f8 = ml_dtypes.float8_e4m3
    x = np.ascontiguousarray(np.asarray(x, dtype=np.float32))
    B = x.shape[0]
    assert B == NCORES and x.shape[1:] == (C, H, W)
    xf = np.zeros((B, C, XLEN), dtype=np.float16)
    view = xf[:, :, HEAD:HEAD + (H + 2 * VPAD) * RS]
    view = view.reshape(B, C, H + 2 * VPAD, RS)
    view[:, :, VPAD:VPAD + H, 0:W] = x.astype(np.float16)
    x8 = xf.astype(f8)
    # margin-free transposed fp8 x for the G^T GEMM: [p, ch, nch, 257]
    # (col 256: 1.0 in ch0 = softmax-denominator ones-column, 0 in ch1)
    xd = x8[:, :, HEAD + VPAD * RS:HEAD + (VPAD + H) * RS]
    xd = xd.reshape(B, C, H, RS)[:, :, :, 0:W].reshape(B, C, H * W)
    xT = np.swapaxes(xd, 1, 2)                        # [B, 4096, C]
    xT = xT.reshape(B, NCH, P, 2, 256)                # [B, nch, p, ch, c]
    xT8 = np.zeros((B, P, 2, NCH, XTW), dtype=f8)
    xT8[:, :, :, :, 0:256] = xT.transpose(0, 2, 3, 1, 4)
    xT8[:, :, 0, :, 256] = f8(1.0)
    xT8 = xT8.reshape(B, P, 2 * NCH * XTW)
    wq8T = np.zeros((C, 16), dtype=f8)
    wq8T[:, 0:9] = np.ascontiguousarray(
        np.asarray(Wq, np.float32).T).astype(f8)
    shared = {
        "wkT": np.ascontiguousarray(np.asarray(Wk, np.float32).T).astype(np.float16),
        "wq8T": wq8T,
        "bk": np.ascontiguousarray(np.asarray(bk, np.float32)),
        "id9h": np.eye(9, dtype=np.float16),
        "id8": np.eye(P, dtype=f8),
    }
    return [dict(shared, xf=np.ascontiguousarray(xf[i]),
                 x8=np.ascontiguousarray(x8[i]), xT8=np.ascontiguousarray(xT8[i]))
            for i in range(B)]


def kernel(x, Wk, bk, Wq, bq):
    from concourse.bass_utils import run_bass_kernel_spmd

    in_maps = make_in_maps(x, Wk, bk, Wq, bq)
    nc = _get_program()
    res = run_bass_kernel_spmd(nc, in_maps, list(range(NCORES))).results
    return np.stack([np.asarray(res[i]["out"], np.float32)
                     for i in range(NCORES)])
